# revision 20
# baseline (speedup 1.0000x reference)
"""Trainium2 Bass kernel for MultiLatentAttention (MLA).

Sharding: 8 cores = 2 (batch) x 4 (head-groups of 4 heads).
Within each batch group of 4 cores, the down-projections are sharded by
output rows and AllGathered (per S-panel, pipelined); the shared k_rope
head is sharded by S-panel and gathered once early.  Each core then runs
its 4 heads' up-projections + SDPA and a partial output projection
y_part = attn_out @ Wo[:, heads].T.  Host sums the 4 partials per batch.

On-device layout is feature-major ("transposed"): activations are [feat, S]
so every matmul contracts along the partition dim with zero transposes.
Scores are computed transposed [k, q]; softmax denominator comes from a
ones-vector matmul; normalization uses a K=1 broadcast matmul.
All matmul operands are bf16 (f32 PSUM accumulation).
"""

import sys

if "/opt/trn_rl_repo" not in sys.path:
    sys.path.insert(0, "/opt/trn_rl_repo")

import numpy as np
import ml_dtypes

BF16 = ml_dtypes.bfloat16

B, S, D, H = 2, 2048, 2048, 16
QR, KVR = 1536, 512
NOPE, RD, VD = 128, 64, 128
QK_D = NOPE + RD
HL = 4          # heads per core
G = 4           # head groups (= cores per batch group)
QSH = QR // G   # 384 c_q rows per core
KSH = KVR // G  # 128 c_kv rows per core
PAN = 512       # panel width
P = 128

_cache = {}


def _build_module(reps=1, phases="ABCD"):
    import concourse.bacc as bacc
    import concourse.mybir as mybir
    import concourse.tile as tile

    dt = mybir.dt
    f32, bf16 = dt.float32, dt.bfloat16
    AF = mybir.ActivationFunctionType

    nc = bacc.Bacc("TRN2", target_bir_lowering=False, debug=False, num_devices=8)

    def inp(name, shape, dtype=bf16):
        return nc.dram_tensor(name, shape, dtype, kind="ExternalInput").ap()

    xT = inp("xT", [D, S])                  # x[b].T
    xkr = inp("xkr", [D, PAN])              # x[b].T[:, my panel]
    wqd = inp("wqd", [D, QSH])              # Wq_down.T column slice
    wkvd = inp("wkvd", [D, KSH])            # Wkv_down.T column slice
    wkr = inp("wkr", [D, RD])               # Wk_rope.T
    wqall = inp("wqall", [QR, 768])         # [Wq_up_g.T*s | Wq_rope_g.T*s]
    wku = inp("wku", [KVR, 512])            # Wk_up_g.T
    wvu = inp("wvu", [KVR, 512])            # Wv_up_g.T
    wo = inp("wo", [512, D])                # Wo[:, cols_g].T
    cosT = inp("cosT", [32, S], f32)
    sinT = inp("sinT", [32, S], f32)
    coskr = inp("coskr", [32, PAN], f32)    # cos/sin for my k_rope panel
    sinkr = inp("sinkr", [32, PAN], f32)
    masks = inp("masks", [P, 4 * PAN])      # multiplicative causal masks
    onc = inp("onc", [P, 1])                # ones column
    y = nc.dram_tensor("y", [S, D], f32, kind="ExternalOutput").ap()

    KT_D = D // P      # 16 k-tiles over model dim
    KT_QR = QR // P    # 12
    KT_KV = KVR // P   # 4
    NP = S // PAN      # 4 panels
    GROUPS = [[0, 1, 2, 3], [4, 5, 6, 7]]

    with tile.TileContext(nc) as tc:
      for _rep in range(reps):
        with (
            tc.tile_pool(name="res", bufs=1) as res,
            tc.tile_pool(name="panels", bufs=10) as panels,
            tc.tile_pool(name="work", bufs=2) as work,
            tc.tile_pool(name="dram", bufs=1, space="DRAM") as dram,
        ):
            # ---- SBUF residents for SDPA --------------------------------
            qn_sb = res.tile([P, HL, S], bf16, tag="qn")
            qr_sb = res.tile([64, HL, S], bf16, tag="qr")
            k_c_sb = res.tile([P, HL, S], bf16, tag="k_c")
            v_sb = res.tile([P, S // P, 512], bf16, tag="v")
            k_r_sb = res.tile([64, NP, PAN], bf16, tag="k_r")
            masks_sb = res.tile([P, G, PAN], bf16, tag="masks")
            onc_sb = res.tile([P, 1], bf16, tag="onc")

            # ---- DRAM staging -------------------------------------------
            ag_in = [dram.tile([QSH + KSH, PAN], bf16, tag=f"agi{n}", name=f"agi{n}")
                     for n in range(NP)]
            ag_out = [dram.tile([G * (QSH + KSH), PAN], bf16, tag=f"ago{n}",
                                name=f"ago{n}") for n in range(NP)]
            kr_in = dram.tile([64, PAN], bf16, tag="kri", name="kri")
            kr_out = dram.tile([G * 64, PAN], bf16, tag="kro", name="kro")
            ao_dram = [dram.tile([HL * P, PAN], bf16, tag=f"aod{g}",
                                 name=f"aod{g}") for g in range(NP)]

            def rope_block(dst64, src64, cs, sn):
                # dst/src are [64, PAN]; rows 0:32 = first half dims
                t1 = work.tile([32, PAN], f32, tag="rope_t1")
                t2 = work.tile([32, PAN], f32, tag="rope_t2")
                nc.vector.tensor_mul(t1, src64[0:32, :], cs)
                nc.vector.tensor_mul(t2, src64[32:64, :], sn)
                nc.vector.tensor_sub(dst64[0:32, :], t1, t2)
                t3 = work.tile([32, PAN], f32, tag="rope_t1")
                t4 = work.tile([32, PAN], f32, tag="rope_t2")
                nc.vector.tensor_mul(t3, src64[32:64, :], cs)
                nc.vector.tensor_mul(t4, src64[0:32, :], sn)
                nc.vector.tensor_add(dst64[32:64, :], t3, t4)

            # ---- Phase A + B, panel-interleaved -------------------------
            with (
                tc.tile_pool(name="pa", bufs=1) as pa,
                tc.tile_pool(name="pb", bufs=1) as pb,
                tc.tile_pool(name="pbc", bufs=2) as pbc,
                tc.tile_pool(name="psA", bufs=3, space="PSUM") as psA,
                tc.tile_pool(name="psB", bufs=3, space="PSUM") as psB,
            ):
                # -- k_rope for my panel first, so the small gather clears early
                wkr_sb = pa.tile([P, KT_D, RD], bf16, tag="wkr")
                nc.sync.dma_start(wkr_sb[:], wkr.rearrange("(kt p) m -> p kt m", p=P))
                xkrr = xkr.rearrange("(c k p) s -> p c k s", p=P, k=4)
                xkr_ch = []
                for c in range(4):
                    t = panels.tile([P, 4, PAN], bf16, tag="panel", name=f"xkr{c}")
                    nc.sync.dma_start(t[:], xkrr[:, c, :, :])
                    xkr_ch.append(t)
                ckr_sb = pa.tile([32, PAN], f32, tag="ckr")
                skr_sb = pa.tile([32, PAN], f32, tag="skr")
                nc.sync.dma_start(ckr_sb[:], coskr[:])
                nc.sync.dma_start(skr_sb[:], sinkr[:])
                ps = psA.tile([64, PAN], f32, tag="psKR", bufs=1)
                for kt in range(KT_D):
                    nc.tensor.matmul(
                        ps, lhsT=wkr_sb[:, kt, :], rhs=xkr_ch[kt // 4][:, kt % 4, :],
                        start=(kt == 0), stop=(kt == KT_D - 1),
                    )
                krst = work.tile([64, PAN], bf16, tag="krst", bufs=1)
                rope_block(krst, ps, ckr_sb, skr_sb)
                nc.sync.dma_start(kr_in[:], krst)
                nc.gpsimd.collective_compute(
                    "AllGather", mybir.AluOpType.bypass,
                    replica_groups=GROUPS,
                    ins=[kr_in.opt()], outs=[kr_out.opt()],
                )
                nc.sync.dma_start(
                    k_r_sb[:], kr_out.rearrange("(g d) s -> d g s", d=64)
                )

                # -- A weights
                wqd_sb = pa.tile([P, KT_D, QSH], bf16, tag="wqd")
                nc.sync.dma_start(wqd_sb[:], wqd.rearrange("(kt p) m -> p kt m", p=P))
                wkvd_sb = pa.tile([P, KT_D, KSH], bf16, tag="wkvd")
                nc.sync.dma_start(wkvd_sb[:], wkvd.rearrange("(kt p) m -> p kt m", p=P))

                def phase_a(n):
                    """my slices of c_q / c_kv for panel n, then gather"""
                    ns = slice(n * PAN, (n + 1) * PAN)
                    xr = xT[:, ns].rearrange("(c k p) s -> p c k s", p=P, k=4)
                    x_ch = []
                    for c in range(4):
                        t = panels.tile([P, 4, PAN], bf16, tag="panel",
                                        name=f"x_sb{n}_{c}")
                        nc.sync.dma_start(t[:], xr[:, c, :, :])
                        x_ch.append(t)
                    for m in range(QSH // P):  # 3 c_q row-tiles
                        ps = psA.tile([P, PAN], f32, tag="psA")
                        for kt in range(KT_D):
                            nc.tensor.matmul(
                                ps,
                                lhsT=wqd_sb[:, kt, m * P : (m + 1) * P],
                                rhs=x_ch[kt // 4][:, kt % 4, :],
                                start=(kt == 0), stop=(kt == KT_D - 1),
                            )
                        st = work.tile([P, PAN], bf16, tag="cq_st")
                        nc.vector.tensor_copy(st, ps)
                        nc.sync.dma_start(ag_in[n][m * P : (m + 1) * P, :], st)
                    ps = psA.tile([P, PAN], f32, tag="psA")  # 1 c_kv row-tile
                    for kt in range(KT_D):
                        nc.tensor.matmul(
                            ps, lhsT=wkvd_sb[:, kt, :], rhs=x_ch[kt // 4][:, kt % 4, :],
                            start=(kt == 0), stop=(kt == KT_D - 1),
                        )
                    st = work.tile([P, PAN], bf16, tag="cq_st")
                    nc.vector.tensor_copy(st, ps)
                    nc.sync.dma_start(ag_in[n][QSH : QSH + KSH, :], st)
                    nc.gpsimd.collective_compute(
                        "AllGather", mybir.AluOpType.bypass,
                        replica_groups=GROUPS,
                        ins=[ag_in[n].opt()], outs=[ag_out[n].opt()],
                    )

                def phase_b(n):
                    """up-projections for panel n from the gathered latents"""
                    ns = slice(n * PAN, (n + 1) * PAN)
                    # gathered latents: [(gi r p), s] with r=0..2 c_q, r=3 c_kv
                    gat = ag_out[n].rearrange("(gi r p) s -> p gi r s", p=P, r=4)
                    cq_ch = []
                    for gi in range(G):
                        t = panels.tile([P, 3, PAN], bf16, tag="panel",
                                        name=f"cq_sb{n}_{gi}")
                        nc.sync.dma_start(t[:], gat[:, gi, 0:3, :])
                        cq_ch.append(t)
                    ckv_sb = pbc.tile([P, KT_KV, PAN], bf16, tag="ckv")
                    nc.sync.dma_start(ckv_sb[:], gat[:, :, 3, :])
                    cosp = pbc.tile([32, PAN], f32, tag="cosp", bufs=1)
                    sinp = pbc.tile([32, PAN], f32, tag="sinp", bufs=1)
                    nc.sync.dma_start(cosp[:], cosT[:, ns])
                    nc.sync.dma_start(sinp[:], sinT[:, ns])
                    for m in range(4):  # q nope heads
                        ps = psB.tile([P, PAN], f32, tag="psB")
                        for kt in range(KT_QR):
                            nc.tensor.matmul(
                                ps,
                                lhsT=wqall_sb[:, kt, m * P : (m + 1) * P],
                                rhs=cq_ch[kt // 3][:, kt % 3, :],
                                start=(kt == 0), stop=(kt == KT_QR - 1),
                            )
                        nc.vector.tensor_copy(qn_sb[:, m, ns], ps)
                    # rope heads: two heads per M=128 matmul; the rope DVE
                    # ops read the psum halves at shifted partition bases
                    for hp in range(HL // 2):
                        c0 = 512 + 128 * hp
                        ps = psB.tile([P, PAN], f32, tag="psB")
                        for kt in range(KT_QR):
                            nc.tensor.matmul(
                                ps,
                                lhsT=wqall_sb[:, kt, c0 : c0 + 128],
                                rhs=cq_ch[kt // 3][:, kt % 3, :],
                                start=(kt == 0), stop=(kt == KT_QR - 1),
                            )
                        rope_block(qr_sb[:, 2 * hp, ns], ps[0:64, :], cosp, sinp)
                        rope_block(qr_sb[:, 2 * hp + 1, ns], ps[64:128, :], cosp, sinp)
                    # k_c for this panel
                    for m in range(HL):
                        ps = psB.tile([P, PAN], f32, tag="psB")
                        for kt in range(KT_KV):
                            nc.tensor.matmul(
                                ps,
                                lhsT=wku_sb[:, kt, m * P : (m + 1) * P],
                                rhs=ckv_sb[:, kt, :],
                                start=(kt == 0), stop=(kt == KT_KV - 1),
                            )
                        nc.vector.tensor_copy(k_c_sb[:, m, ns], ps)
                    # v for this panel's S-tiles
                    for sti in range(4):
                        st = 4 * n + sti
                        ps = psB.tile([P, PAN], f32, tag="psB")
                        for kt in range(KT_KV):
                            nc.tensor.matmul(
                                ps,
                                lhsT=ckv_sb[:, kt, sti * P : (sti + 1) * P],
                                rhs=wvu_sb[:, kt, :],
                                start=(kt == 0), stop=(kt == KT_KV - 1),
                            )
                        nc.vector.tensor_copy(v_sb[:, st, :], ps)

                # interleave emission so the shared panel slots rotate A/B/A/B
                phase_a(0)
                # -- B weights (gpsimd DMA queue, off the hot SP queue)
                wqall_sb = pb.tile([P, KT_QR, 768], bf16, tag="wqall")
                nc.gpsimd.dma_start(wqall_sb[:], wqall.rearrange("(kt p) m -> p kt m", p=P))
                wku_sb = pb.tile([P, KT_KV, 512], bf16, tag="wku")
                nc.gpsimd.dma_start(wku_sb[:], wku.rearrange("(kt p) m -> p kt m", p=P))
                wvu_sb = pb.tile([P, KT_KV, 512], bf16, tag="wvu")
                nc.gpsimd.dma_start(wvu_sb[:], wvu.rearrange("(kt p) m -> p kt m", p=P))
                phase_a(1)
                phase_b(0)
                phase_a(2)
                phase_b(1)
                phase_a(3)
                phase_b(2)
                phase_b(3)

            # ---------------- Phase C: SDPA + Phase D interleaved --------
            if "C" not in phases:
                # timing-partial build: consume B outputs so nothing is elided
                nc.gpsimd.dma_start(y[0:P, 0:PAN], qn_sb[:, 0, 0:PAN])
                nc.gpsimd.dma_start(y[P : 2 * P, 0:PAN], k_c_sb[:, 0, 0:PAN])
                nc.gpsimd.dma_start(y[2 * P : 3 * P, 0:PAN], v_sb[:, 0, 0:PAN])
                nc.gpsimd.dma_start(y[3 * P : 3 * P + 64, 0:PAN], qr_sb[:, 0, 0:PAN])
                nc.gpsimd.dma_start(y[4 * P : 4 * P + 64, 0:PAN], k_r_sb[:, 0, :])
                continue
            with (
                tc.tile_pool(name="pe", bufs=4) as pe,
                tc.tile_pool(name="pd", bufs=1) as pd,
                tc.tile_pool(name="pda", bufs=2) as pda,
                tc.tile_pool(name="psS", bufs=3, space="PSUM") as psS,
                tc.tile_pool(name="psO", bufs=2, space="PSUM") as psO,
                tc.tile_pool(name="psDn", bufs=2, space="PSUM") as psDn,
                tc.tile_pool(name="psD", bufs=1, space="PSUM") as psD,
            ):
                nc.gpsimd.dma_start(
                    masks_sb[:], masks.rearrange("p (j q) -> p j q", q=PAN)
                )
                nc.gpsimd.dma_start(onc_sb[:], onc[:])
                wo_sb = pd.tile([P, HL, D], bf16, tag="wo")
                nc.gpsimd.dma_start(wo_sb[:], wo.rearrange("(kt p) m -> p kt m", p=P))

                def phase_d(m):
                    if "D" not in phases:
                        return
                    ms = slice(m * P, (m + 1) * P)
                    g = m // 4
                    aog = ao_dram[g].rearrange("(h p) s -> p h s", p=P)
                    ao_sb = pda.tile([P, HL, P], bf16, tag="ao_rd")
                    nc.sync.dma_start(
                        ao_sb[:], aog[:, :, (m % 4) * P : (m % 4 + 1) * P]
                    )
                    for nn in range(D // PAN):
                        ps = psD.tile([P, PAN], f32, tag="psD")
                        for kt in range(HL):
                            nc.tensor.matmul(
                                ps,
                                lhsT=ao_sb[:, kt, :],
                                rhs=wo_sb[:, kt, nn * PAN : (nn + 1) * PAN],
                                start=(kt == 0), stop=(kt == HL - 1),
                            )
                        yst = work.tile([P, PAN], f32, tag="y_st")
                        nc.vector.tensor_copy(yst, ps)
                        nc.sync.dma_start(y[ms, nn * PAN : (nn + 1) * PAN], yst)

                for g in range(G):
                    gs = slice(g * PAN, (g + 1) * PAN)
                    for h in range(HL):
                        qn = qn_sb[:, h, gs]
                        qr = qr_sb[:, h, gs]
                        ps_o = psO.tile([P, PAN], f32, tag="ps_o")
                        ps_d = psDn.tile([1, PAN], f32, tag="ps_d")
                        nk = 4 * (g + 1)
                        for kb in range(nk):
                            ks = slice(kb * P, (kb + 1) * P)
                            ps_s = psS.tile([P, PAN], f32, tag="ps_s")
                            nc.tensor.matmul(
                                ps_s, lhsT=k_c_sb[:, h, ks], rhs=qn,
                                start=True, stop=False,
                            )
                            nc.tensor.matmul(
                                ps_s,
                                lhsT=k_r_sb[:, kb // 4, (kb % 4) * P : (kb % 4 + 1) * P],
                                rhs=qr,
                                start=False, stop=True,
                            )
                            e = pe.tile([P, PAN], bf16, tag="e")
                            nc.scalar.activation(e, ps_s, AF.Exp)
                            if kb >= 4 * g:
                                nc.vector.tensor_mul(
                                    e, e, masks_sb[:, kb - 4 * g, :]
                                )
                            nc.tensor.matmul(
                                ps_o, lhsT=v_sb[:, kb, h * P : (h + 1) * P], rhs=e,
                                start=(kb == 0), stop=(kb == nk - 1),
                            )
                            nc.tensor.matmul(
                                ps_d, lhsT=onc_sb[:], rhs=e,
                                start=(kb == 0), stop=(kb == nk - 1),
                            )
                        rc = work.tile([1, PAN], f32, tag="rc")
                        nc.vector.reciprocal(rc, ps_d)
                        bb = work.tile([P, PAN], f32, tag="bb")
                        nc.gpsimd.partition_broadcast(bb, rc)
                        ao_st = work.tile([P, PAN], bf16, tag="ao_st")
                        nc.vector.tensor_mul(ao_st, ps_o, bb)
                        nc.sync.dma_start(ao_dram[g][h * P : (h + 1) * P, :], ao_st)
                    for m in range(4 * g, 4 * g + 4):
                        phase_d(m)

    nc.compile()
    return nc


def _prep_inputs(x, positions, Wq_down, Wq_up, Wq_rope, Wkv_down, Wk_up, Wv_up,
                 Wk_rope, Wo):
    scale = np.float32(QK_D ** -0.5)
    bf = lambda a: np.ascontiguousarray(a).astype(BF16)

    shared = {
        "wkr": bf(Wk_rope.T),
        "onc": np.ones((P, 1), BF16),
    }
    inv_freq = 1.0 / (10000.0 ** (np.arange(0, RD, 2, dtype=np.float32) / RD))
    ang = positions.astype(np.float32)[:, None] * inv_freq  # (S, 32)
    cosT = np.ascontiguousarray(np.cos(ang).T).astype(np.float32)
    sinT = np.ascontiguousarray(np.sin(ang).T).astype(np.float32)
    shared["cosT"] = cosT
    shared["sinT"] = sinT

    mk = np.zeros((P, G * PAN), np.float32)
    for j in range(G):
        p = np.arange(P)[:, None]
        q = np.arange(PAN)[None, :]
        mk[:, j * PAN : (j + 1) * PAN] = (j * P + p <= q).astype(np.float32)
    shared["masks"] = mk.astype(BF16)

    wqdT = Wq_down.T  # (D, QR)
    wkvdT = Wkv_down.T  # (D, KVR)
    per_g = []
    for g in range(G):
        rs, rr = slice(512 * g, 512 * (g + 1)), slice(256 * g, 256 * (g + 1))
        per_g.append({
            "wqd": bf(wqdT[:, QSH * g : QSH * (g + 1)]),
            "wkvd": bf(wkvdT[:, KSH * g : KSH * (g + 1)]),
            "wqall": bf(np.concatenate(
                [(Wq_up[rs] * scale).T, (Wq_rope[rr] * scale).T], axis=1)),
            "wku": bf(Wk_up[rs].T),
            "wvu": bf(Wv_up[rs].T),
            "wo": bf(Wo[:, rs].T),
            "coskr": np.ascontiguousarray(cosT[:, PAN * g : PAN * (g + 1)]),
            "sinkr": np.ascontiguousarray(sinT[:, PAN * g : PAN * (g + 1)]),
        })
    xT = [bf(x[b].T) for b in range(B)]

    in_maps = []
    for c in range(8):
        b, g = c // G, c % G
        m = dict(shared)
        m.update(per_g[g])
        m["xT"] = xT[b]
        m["xkr"] = np.ascontiguousarray(xT[b][:, PAN * g : PAN * (g + 1)])
        in_maps.append(m)
    return in_maps


def kernel(**inputs):
    from concourse.bass_utils import run_bass_kernel_spmd

    if "nc" not in _cache:
        _cache["nc"] = _build_module()
    nc = _cache["nc"]

    in_maps = _prep_inputs(**inputs)
    res = run_bass_kernel_spmd(nc, in_maps, core_ids=list(range(8)))
    out = np.zeros((B, S, D), np.float32)
    for c in range(8):
        out[c // G] += res.results[c]["y"]
    return out


# revision 21
# speedup vs baseline: 183.3232x; 183.3232x over previous
"""Trainium2 Bass kernel for MultiLatentAttention (MLA).

Sharding: 8 cores = 2 (batch) x 4 (head-groups of 4 heads).
Within each batch group of 4 cores, the down-projections are sharded by
output rows and AllGathered (per S-panel, pipelined); the shared k_rope
head is sharded by S-panel and gathered once early.  Each core then runs
its 4 heads' up-projections + SDPA and a partial output projection
y_part = attn_out @ Wo[:, heads].T.  Host sums the 4 partials per batch.

On-device layout is feature-major ("transposed"): activations are [feat, S]
so every matmul contracts along the partition dim with zero transposes.
Scores are computed transposed [k, q]; softmax denominator comes from a
ones-vector matmul; normalization uses a K=1 broadcast matmul.
All matmul operands are bf16 (f32 PSUM accumulation).
"""

import sys

if "/opt/trn_rl_repo" not in sys.path:
    sys.path.insert(0, "/opt/trn_rl_repo")

import numpy as np
import ml_dtypes

BF16 = ml_dtypes.bfloat16

B, S, D, H = 2, 2048, 2048, 16
QR, KVR = 1536, 512
NOPE, RD, VD = 128, 64, 128
QK_D = NOPE + RD
HL = 4          # heads per core
G = 4           # head groups (= cores per batch group)
QSH = QR // G   # 384 c_q rows per core
KSH = KVR // G  # 128 c_kv rows per core
PAN = 512       # panel width
P = 128

_cache = {}


def _build_module(reps=1, phases="ABCD"):
    import concourse.bacc as bacc
    import concourse.mybir as mybir
    import concourse.tile as tile

    dt = mybir.dt
    f32, bf16 = dt.float32, dt.bfloat16
    AF = mybir.ActivationFunctionType

    nc = bacc.Bacc("TRN2", target_bir_lowering=False, debug=False, num_devices=8)

    def inp(name, shape, dtype=bf16):
        return nc.dram_tensor(name, shape, dtype, kind="ExternalInput").ap()

    xT = inp("xT", [D, S])                  # x[b].T
    xkr = inp("xkr", [D, PAN])              # x[b].T[:, my panel]
    wqd = inp("wqd", [D, QSH])              # Wq_down.T column slice
    wkvd = inp("wkvd", [D, KSH])            # Wkv_down.T column slice
    wkr = inp("wkr", [D, RD])               # Wk_rope.T
    wqall = inp("wqall", [QR, 768])         # [Wq_up_g.T*s | Wq_rope_g.T*s]
    wku = inp("wku", [KVR, 512])            # Wk_up_g.T
    wvu = inp("wvu", [KVR, 512])            # Wv_up_g.T
    wo = inp("wo", [512, D])                # Wo[:, cols_g].T
    cosT = inp("cosT", [32, S], f32)
    sinT = inp("sinT", [32, S], f32)
    coskr = inp("coskr", [32, PAN], f32)    # cos/sin for my k_rope panel
    sinkr = inp("sinkr", [32, PAN], f32)
    masks = inp("masks", [P, 4 * PAN])      # multiplicative causal masks
    onc = inp("onc", [P, 1])                # ones column
    y = nc.dram_tensor("y", [S, D], f32, kind="ExternalOutput").ap()

    KT_D = D // P      # 16 k-tiles over model dim
    KT_QR = QR // P    # 12
    KT_KV = KVR // P   # 4
    NP = S // PAN      # 4 panels
    GROUPS = [[0, 1, 2, 3], [4, 5, 6, 7]]

    with tile.TileContext(nc) as tc:
      for _rep in range(reps):
        with (
            tc.tile_pool(name="res", bufs=1) as res,
            tc.tile_pool(name="panels", bufs=10) as panels,
            tc.tile_pool(name="work", bufs=2) as work,
            tc.tile_pool(name="dram", bufs=1, space="DRAM") as dram,
        ):
            # ---- SBUF residents for SDPA --------------------------------
            qn_sb = res.tile([P, HL, S], bf16, tag="qn")
            qr_sb = res.tile([64, HL, S], bf16, tag="qr")
            k_c_sb = res.tile([P, HL, S], bf16, tag="k_c")
            v_sb = res.tile([P, S // P, 512], bf16, tag="v")
            k_r_sb = res.tile([64, NP, PAN], bf16, tag="k_r")
            masks_sb = res.tile([P, G, PAN], bf16, tag="masks")
            onc_sb = res.tile([P, 1], bf16, tag="onc")

            # ---- DRAM staging -------------------------------------------
            ag_in = [dram.tile([QSH + KSH, PAN], bf16, tag=f"agi{n}", name=f"agi{n}")
                     for n in range(NP)]
            ag_out = [dram.tile([G * (QSH + KSH), PAN], bf16, tag=f"ago{n}",
                                name=f"ago{n}") for n in range(NP)]
            kr_in = dram.tile([64, PAN], bf16, tag="kri", name="kri")
            kr_out = dram.tile([G * 64, PAN], bf16, tag="kro", name="kro")
            ao_dram = [dram.tile([HL * P, PAN], bf16, tag=f"aod{g}",
                                 name=f"aod{g}") for g in range(NP)]

            def rope_block(dst64, src64, cs, sn):
                # dst/src are [64, PAN]; rows 0:32 = first half dims
                t1 = work.tile([32, PAN], f32, tag="rope_t1")
                t2 = work.tile([32, PAN], f32, tag="rope_t2")
                nc.vector.tensor_mul(t1, src64[0:32, :], cs)
                nc.vector.tensor_mul(t2, src64[32:64, :], sn)
                nc.vector.tensor_sub(dst64[0:32, :], t1, t2)
                t3 = work.tile([32, PAN], f32, tag="rope_t1")
                t4 = work.tile([32, PAN], f32, tag="rope_t2")
                nc.vector.tensor_mul(t3, src64[32:64, :], cs)
                nc.vector.tensor_mul(t4, src64[0:32, :], sn)
                nc.vector.tensor_add(dst64[32:64, :], t3, t4)

            # ---- Phase A + B, panel-interleaved -------------------------
            with (
                tc.tile_pool(name="pa", bufs=1) as pa,
                tc.tile_pool(name="pb", bufs=1) as pb,
                tc.tile_pool(name="pbc", bufs=2) as pbc,
                tc.tile_pool(name="psA", bufs=3, space="PSUM") as psA,
                tc.tile_pool(name="psB", bufs=3, space="PSUM") as psB,
            ):
                # -- k_rope for my panel first, so the small gather clears early
                wkr_sb = pa.tile([P, KT_D, RD], bf16, tag="wkr")
                nc.sync.dma_start(wkr_sb[:], wkr.rearrange("(kt p) m -> p kt m", p=P))
                xkrr = xkr.rearrange("(c k p) s -> p c k s", p=P, k=4)
                xkr_ch = []
                for c in range(4):
                    t = panels.tile([P, 4, PAN], bf16, tag="panel", name=f"xkr{c}")
                    nc.sync.dma_start(t[:], xkrr[:, c, :, :])
                    xkr_ch.append(t)
                ckr_sb = pa.tile([32, PAN], f32, tag="ckr")
                skr_sb = pa.tile([32, PAN], f32, tag="skr")
                nc.sync.dma_start(ckr_sb[:], coskr[:])
                nc.sync.dma_start(skr_sb[:], sinkr[:])
                ps = psA.tile([64, PAN], f32, tag="psKR", bufs=1)
                for kt in range(KT_D):
                    nc.tensor.matmul(
                        ps, lhsT=wkr_sb[:, kt, :], rhs=xkr_ch[kt // 4][:, kt % 4, :],
                        start=(kt == 0), stop=(kt == KT_D - 1),
                    )
                krst = work.tile([64, PAN], bf16, tag="krst", bufs=1)
                rope_block(krst, ps, ckr_sb, skr_sb)
                nc.sync.dma_start(kr_in[:], krst)
                nc.gpsimd.collective_compute(
                    "AllGather", mybir.AluOpType.bypass,
                    replica_groups=GROUPS,
                    ins=[kr_in.opt()], outs=[kr_out.opt()],
                )
                nc.sync.dma_start(
                    k_r_sb[:], kr_out.rearrange("(g d) s -> d g s", d=64)
                )

                # -- A weights
                wqd_sb = pa.tile([P, KT_D, QSH], bf16, tag="wqd")
                nc.sync.dma_start(wqd_sb[:], wqd.rearrange("(kt p) m -> p kt m", p=P))
                wkvd_sb = pa.tile([P, KT_D, KSH], bf16, tag="wkvd")
                nc.sync.dma_start(wkvd_sb[:], wkvd.rearrange("(kt p) m -> p kt m", p=P))

                def phase_a(n):
                    """my slices of c_q / c_kv for panel n, then gather"""
                    ns = slice(n * PAN, (n + 1) * PAN)
                    xr = xT[:, ns].rearrange("(c k p) s -> p c k s", p=P, k=4)
                    x_ch = []
                    for c in range(4):
                        t = panels.tile([P, 4, PAN], bf16, tag="panel",
                                        name=f"x_sb{n}_{c}")
                        nc.sync.dma_start(t[:], xr[:, c, :, :])
                        x_ch.append(t)
                    for m in range(QSH // P):  # 3 c_q row-tiles
                        ps = psA.tile([P, PAN], f32, tag="psA")
                        for kt in range(KT_D):
                            nc.tensor.matmul(
                                ps,
                                lhsT=wqd_sb[:, kt, m * P : (m + 1) * P],
                                rhs=x_ch[kt // 4][:, kt % 4, :],
                                start=(kt == 0), stop=(kt == KT_D - 1),
                            )
                        st = work.tile([P, PAN], bf16, tag="cq_st")
                        nc.vector.tensor_copy(st, ps)
                        nc.sync.dma_start(ag_in[n][m * P : (m + 1) * P, :], st)
                    ps = psA.tile([P, PAN], f32, tag="psA")  # 1 c_kv row-tile
                    for kt in range(KT_D):
                        nc.tensor.matmul(
                            ps, lhsT=wkvd_sb[:, kt, :], rhs=x_ch[kt // 4][:, kt % 4, :],
                            start=(kt == 0), stop=(kt == KT_D - 1),
                        )
                    st = work.tile([P, PAN], bf16, tag="cq_st")
                    nc.vector.tensor_copy(st, ps)
                    nc.sync.dma_start(ag_in[n][QSH : QSH + KSH, :], st)
                    nc.gpsimd.collective_compute(
                        "AllGather", mybir.AluOpType.bypass,
                        replica_groups=GROUPS,
                        ins=[ag_in[n].opt()], outs=[ag_out[n].opt()],
                    )

                def phase_b(n):
                    """up-projections for panel n from the gathered latents"""
                    ns = slice(n * PAN, (n + 1) * PAN)
                    # gathered latents: [(gi r p), s] with r=0..2 c_q, r=3 c_kv
                    gat = ag_out[n].rearrange("(gi r p) s -> p gi r s", p=P, r=4)
                    cq_ch = []
                    for gi in range(G):
                        t = panels.tile([P, 3, PAN], bf16, tag="panel",
                                        name=f"cq_sb{n}_{gi}")
                        nc.sync.dma_start(t[:], gat[:, gi, 0:3, :])
                        cq_ch.append(t)
                    ckv_sb = pbc.tile([P, KT_KV, PAN], bf16, tag="ckv")
                    nc.sync.dma_start(ckv_sb[:], gat[:, :, 3, :])
                    cosp = pbc.tile([32, PAN], f32, tag="cosp", bufs=1)
                    sinp = pbc.tile([32, PAN], f32, tag="sinp", bufs=1)
                    nc.sync.dma_start(cosp[:], cosT[:, ns])
                    nc.sync.dma_start(sinp[:], sinT[:, ns])
                    for m in range(4):  # q nope heads
                        ps = psB.tile([P, PAN], f32, tag="psB")
                        for kt in range(KT_QR):
                            nc.tensor.matmul(
                                ps,
                                lhsT=wqall_sb[:, kt, m * P : (m + 1) * P],
                                rhs=cq_ch[kt // 3][:, kt % 3, :],
                                start=(kt == 0), stop=(kt == KT_QR - 1),
                            )
                        nc.vector.tensor_copy(qn_sb[:, m, ns], ps)
                    # rope heads: two heads per M=128 matmul; the rope DVE
                    # ops read the psum halves at shifted partition bases
                    for hp in range(HL // 2):
                        c0 = 512 + 128 * hp
                        ps = psB.tile([P, PAN], f32, tag="psB")
                        for kt in range(KT_QR):
                            nc.tensor.matmul(
                                ps,
                                lhsT=wqall_sb[:, kt, c0 : c0 + 128],
                                rhs=cq_ch[kt // 3][:, kt % 3, :],
                                start=(kt == 0), stop=(kt == KT_QR - 1),
                            )
                        rope_block(qr_sb[:, 2 * hp, ns], ps[0:64, :], cosp, sinp)
                        rope_block(qr_sb[:, 2 * hp + 1, ns], ps[64:128, :], cosp, sinp)
                    # k_c for this panel
                    for m in range(HL):
                        ps = psB.tile([P, PAN], f32, tag="psB")
                        for kt in range(KT_KV):
                            nc.tensor.matmul(
                                ps,
                                lhsT=wku_sb[:, kt, m * P : (m + 1) * P],
                                rhs=ckv_sb[:, kt, :],
                                start=(kt == 0), stop=(kt == KT_KV - 1),
                            )
                        nc.vector.tensor_copy(k_c_sb[:, m, ns], ps)
                    # v for this panel's S-tiles
                    for sti in range(4):
                        st = 4 * n + sti
                        ps = psB.tile([P, PAN], f32, tag="psB")
                        for kt in range(KT_KV):
                            nc.tensor.matmul(
                                ps,
                                lhsT=ckv_sb[:, kt, sti * P : (sti + 1) * P],
                                rhs=wvu_sb[:, kt, :],
                                start=(kt == 0), stop=(kt == KT_KV - 1),
                            )
                        nc.vector.tensor_copy(v_sb[:, st, :], ps)

                # interleave emission so the shared panel slots rotate A/B/A/B
                phase_a(0)
                # -- B weights (gpsimd DMA queue, off the hot SP queue)
                wqall_sb = pb.tile([P, KT_QR, 768], bf16, tag="wqall")
                nc.gpsimd.dma_start(wqall_sb[:], wqall.rearrange("(kt p) m -> p kt m", p=P))
                wku_sb = pb.tile([P, KT_KV, 512], bf16, tag="wku")
                nc.gpsimd.dma_start(wku_sb[:], wku.rearrange("(kt p) m -> p kt m", p=P))
                wvu_sb = pb.tile([P, KT_KV, 512], bf16, tag="wvu")
                nc.gpsimd.dma_start(wvu_sb[:], wvu.rearrange("(kt p) m -> p kt m", p=P))
                phase_a(1)
                phase_b(0)
                phase_a(2)
                phase_b(1)
                phase_a(3)
                phase_b(2)
                phase_b(3)

            # ---------------- Phase C: SDPA + Phase D interleaved --------
            if "C" not in phases:
                # timing-partial build: consume B outputs so nothing is elided
                nc.gpsimd.dma_start(y[0:P, 0:PAN], qn_sb[:, 0, 0:PAN])
                nc.gpsimd.dma_start(y[P : 2 * P, 0:PAN], k_c_sb[:, 0, 0:PAN])
                nc.gpsimd.dma_start(y[2 * P : 3 * P, 0:PAN], v_sb[:, 0, 0:PAN])
                nc.gpsimd.dma_start(y[3 * P : 3 * P + 64, 0:PAN], qr_sb[:, 0, 0:PAN])
                nc.gpsimd.dma_start(y[4 * P : 4 * P + 64, 0:PAN], k_r_sb[:, 0, :])
                continue
            with (
                tc.tile_pool(name="pe", bufs=4) as pe,
                tc.tile_pool(name="pd", bufs=1) as pd,
                tc.tile_pool(name="pda", bufs=2) as pda,
                tc.tile_pool(name="psS", bufs=3, space="PSUM") as psS,
                tc.tile_pool(name="psO", bufs=2, space="PSUM") as psO,
                tc.tile_pool(name="psDn", bufs=2, space="PSUM") as psDn,
                tc.tile_pool(name="psD", bufs=1, space="PSUM") as psD,
            ):
                nc.gpsimd.dma_start(
                    masks_sb[:], masks.rearrange("p (j q) -> p j q", q=PAN)
                )
                nc.gpsimd.dma_start(onc_sb[:], onc[:])
                wo_sb = pd.tile([P, HL, D], bf16, tag="wo")
                nc.gpsimd.dma_start(wo_sb[:], wo.rearrange("(kt p) m -> p kt m", p=P))

                def phase_d(m):
                    if "D" not in phases:
                        return
                    ms = slice(m * P, (m + 1) * P)
                    g = m // 4
                    aog = ao_dram[g].rearrange("(h p) s -> p h s", p=P)
                    ao_sb = pda.tile([P, HL, P], bf16, tag="ao_rd")
                    nc.sync.dma_start(
                        ao_sb[:], aog[:, :, (m % 4) * P : (m % 4 + 1) * P]
                    )
                    for nn in range(D // PAN):
                        ps = psD.tile([P, PAN], f32, tag="psD")
                        for kt in range(HL):
                            nc.tensor.matmul(
                                ps,
                                lhsT=ao_sb[:, kt, :],
                                rhs=wo_sb[:, kt, nn * PAN : (nn + 1) * PAN],
                                start=(kt == 0), stop=(kt == HL - 1),
                            )
                        yst = work.tile([P, PAN], f32, tag="y_st")
                        nc.vector.tensor_copy(yst, ps)
                        nc.sync.dma_start(y[ms, nn * PAN : (nn + 1) * PAN], yst)

                for g in range(G):
                    gs = slice(g * PAN, (g + 1) * PAN)
                    for h in range(HL):
                        qn = qn_sb[:, h, gs]
                        qr = qr_sb[:, h, gs]
                        ps_o = psO.tile([P, PAN], f32, tag="ps_o")
                        ps_d = psDn.tile([1, PAN], f32, tag="ps_d")
                        nk = 4 * (g + 1)
                        for kb in range(nk):
                            ks = slice(kb * P, (kb + 1) * P)
                            ps_s = psS.tile([P, PAN], f32, tag="ps_s")
                            nc.tensor.matmul(
                                ps_s, lhsT=k_c_sb[:, h, ks], rhs=qn,
                                start=True, stop=False,
                            )
                            nc.tensor.matmul(
                                ps_s,
                                lhsT=k_r_sb[:, kb // 4, (kb % 4) * P : (kb % 4 + 1) * P],
                                rhs=qr,
                                start=False, stop=True,
                            )
                            e = pe.tile([P, PAN], bf16, tag="e")
                            nc.scalar.activation(e, ps_s, AF.Exp)
                            if kb >= 4 * g:
                                nc.vector.tensor_mul(
                                    e, e, masks_sb[:, kb - 4 * g, :]
                                )
                            nc.tensor.matmul(
                                ps_o, lhsT=v_sb[:, kb, h * P : (h + 1) * P], rhs=e,
                                start=(kb == 0), stop=(kb == nk - 1),
                            )
                            nc.tensor.matmul(
                                ps_d, lhsT=onc_sb[:], rhs=e,
                                start=(kb == 0), stop=(kb == nk - 1),
                            )
                        rc = work.tile([1, PAN], f32, tag="rc")
                        nc.vector.reciprocal(rc, ps_d)
                        bb = work.tile([P, PAN], f32, tag="bb")
                        nc.gpsimd.partition_broadcast(bb, rc)
                        ao_st = work.tile([P, PAN], bf16, tag="ao_st")
                        nc.vector.tensor_mul(ao_st, ps_o, bb)
                        nc.sync.dma_start(ao_dram[g][h * P : (h + 1) * P, :], ao_st)
                    for m in range(4 * g, 4 * g + 4):
                        phase_d(m)

    nc.compile()
    return nc


def _prep_inputs(x, positions, Wq_down, Wq_up, Wq_rope, Wkv_down, Wk_up, Wv_up,
                 Wk_rope, Wo):
    scale = np.float32(QK_D ** -0.5)
    bf = lambda a: np.ascontiguousarray(a).astype(BF16)

    shared = {
        "wkr": bf(Wk_rope.T),
        "onc": np.ones((P, 1), BF16),
    }
    inv_freq = 1.0 / (10000.0 ** (np.arange(0, RD, 2, dtype=np.float32) / RD))
    ang = positions.astype(np.float32)[:, None] * inv_freq  # (S, 32)
    cosT = np.ascontiguousarray(np.cos(ang).T).astype(np.float32)
    sinT = np.ascontiguousarray(np.sin(ang).T).astype(np.float32)
    shared["cosT"] = cosT
    shared["sinT"] = sinT

    mk = np.zeros((P, G * PAN), np.float32)
    for j in range(G):
        p = np.arange(P)[:, None]
        q = np.arange(PAN)[None, :]
        mk[:, j * PAN : (j + 1) * PAN] = (j * P + p <= q).astype(np.float32)
    shared["masks"] = mk.astype(BF16)

    wqdT = Wq_down.T  # (D, QR)
    wkvdT = Wkv_down.T  # (D, KVR)
    per_g = []
    for g in range(G):
        rs, rr = slice(512 * g, 512 * (g + 1)), slice(256 * g, 256 * (g + 1))
        per_g.append({
            "wqd": bf(wqdT[:, QSH * g : QSH * (g + 1)]),
            "wkvd": bf(wkvdT[:, KSH * g : KSH * (g + 1)]),
            "wqall": bf(np.concatenate(
                [(Wq_up[rs] * scale).T, (Wq_rope[rr] * scale).T], axis=1)),
            "wku": bf(Wk_up[rs].T),
            "wvu": bf(Wv_up[rs].T),
            "wo": bf(Wo[:, rs].T),
            "coskr": np.ascontiguousarray(cosT[:, PAN * g : PAN * (g + 1)]),
            "sinkr": np.ascontiguousarray(sinT[:, PAN * g : PAN * (g + 1)]),
        })
    xT = [bf(x[b].T) for b in range(B)]

    in_maps = []
    for c in range(8):
        b, g = c // G, c % G
        m = dict(shared)
        m.update(per_g[g])
        m["xT"] = xT[b]
        m["xkr"] = np.ascontiguousarray(xT[b][:, PAN * g : PAN * (g + 1)])
        in_maps.append(m)
    return in_maps


def kernel(**inputs):
    from concourse.bass_utils import run_bass_kernel_spmd

    if "nc" not in _cache:
        _cache["nc"] = _build_module()
    nc = _cache["nc"]

    in_maps = _prep_inputs(**inputs)
    res = None
    for attempt in range(3):
        try:
            res = run_bass_kernel_spmd(nc, in_maps, core_ids=list(range(8)))
            break
        except Exception:
            if attempt == 2:
                raise
    out = np.zeros((B, S, D), np.float32)
    for c in range(8):
        out[c // G] += res.results[c]["y"]
    return out
